# revision 6
# baseline (speedup 1.0000x reference)
"""ClockHConvGRUCell on 8 Trainium2 NeuronCores — data-parallel over batch.

Contract: kernel(**inputs) takes the FULL unsharded inputs (numpy), returns
(inhibition_new, excitation_new) with full shapes [8,128,96,96] f32.

Per-core layout: channels (H=128) on SBUF partitions, spatial (96*96=9216) on
the free dim.  1x1 convs are 128x128 matmuls over 384-column chunks; the 5x5
convs are 25 shifted matmuls accumulated in PSUM over a zero-padded bf16
[100x100] image.  BatchNorm batch-stats are AllReduce'd across the 8 cores.
"""

import math
import sys

sys.path.insert(0, "/opt/trn_rl_repo")

import numpy as np

import concourse.bacc as bacc
import concourse.bass as bass
import concourse.tile as tile
from concourse import mybir
from concourse.bass_utils import run_bass_kernel_spmd

F32 = mybir.dt.float32
BF16 = mybir.dt.bfloat16
AF = mybir.ActivationFunctionType
OP = mybir.AluOpType

H = 128
S = 96
SS = S * S          # 9216
W = S + 4           # padded width/height (2 halo each side)
NCORES = 8
CHR = 4             # output rows per chunk
NCH = S // CHR      # 24 chunks
CHW = CHR * S       # 384 columns per chunk
CGRP = 4            # conv chunks per weight sweep (4 psum banks)
PI = math.pi
EPS = 1e-3
NTOT = float(NCORES * SS)  # batchnorm population count


def _build_program():
    nc = bacc.Bacc("TRN2", target_bir_lowering=False, debug=False,
                   num_devices=NCORES)

    x_d = nc.dram_tensor("x", [H, SS], F32, kind="ExternalInput").ap()
    inh_d = nc.dram_tensor("inh", [H, SS], F32, kind="ExternalInput").ap()
    exc_d = nc.dram_tensor("exc", [H, SS], F32, kind="ExternalInput").ap()
    w1_d = nc.dram_tensor("w1x1", [9, H, H], F32, kind="ExternalInput").ap()
    w5_d = nc.dram_tensor("w5", [2, 25, H, H], F32, kind="ExternalInput").ap()
    par_d = nc.dram_tensor("params", [H, 16], F32, kind="ExternalInput").ap()
    oinh_d = nc.dram_tensor("out_inh", [H, SS], F32, kind="ExternalOutput").ap()
    oexc_d = nc.dram_tensor("out_exc", [H, SS], F32, kind="ExternalOutput").ap()

    # params columns
    C_BATT, C_BCA, C_BINH, C_BCI, C_BEXC, C_BCE = 0, 1, 2, 3, 4, 5
    C_ALPHA, C_MU, C_KAPPA, C_GAMMA, C_WGAIN = 6, 7, 8, 9, 10
    C_BN0W, C_BN0B, C_BN1W, C_BN1B = 11, 12, 13, 14

    from contextlib import ExitStack
    with tile.TileContext(nc) as tc, ExitStack() as ctx:
        const = ctx.enter_context(tc.tile_pool(name="const", bufs=1))
        wpool = ctx.enter_context(tc.tile_pool(name="wpool", bufs=2))
        xs = ctx.enter_context(tc.tile_pool(name="xs", bufs=3))
        sc = ctx.enter_context(tc.tile_pool(name="sc", bufs=3))
        stp = ctx.enter_context(tc.tile_pool(name="stp", bufs=1))
        pp = ctx.enter_context(tc.tile_pool(name="pp", bufs=2, space="PSUM"))
        dp = ctx.enter_context(tc.tile_pool(name="dp", bufs=2, space="DRAM"))

        par = const.tile([H, 16], F32, name="par")
        nc.sync.dma_start(out=par, in_=par_d)
        eps_sb = const.tile([H, 1], F32, name="eps_sb")
        nc.vector.memset(eps_sb, EPS)

        inh_sb = const.tile([H, SS], F32, name="inh_sb")
        exc_sb = const.tile([H, SS], F32, name="exc_sb")
        intx = const.tile([H, SS], F32, name="intx")
        pad = const.tile([H, W, W], BF16, name="pad")
        nc.vector.memset(pad, 0.0)
        # split resident loads so early chunks can start sooner
        for q in range(4):
            c0 = q * (SS // 4)
            nc.sync.dma_start(out=exc_sb[:, c0:c0 + SS // 4],
                              in_=exc_d[:, c0:c0 + SS // 4])
        for q in range(4):
            c0 = q * (SS // 4)
            nc.sync.dma_start(out=inh_sb[:, c0:c0 + SS // 4],
                              in_=inh_d[:, c0:c0 + SS // 4])

        # stage-A 1x1 weights (aw, au, ac)
        wA = wpool.tile([H, 3, H], F32, tag="w1", name="wA")
        nc.sync.dma_start(out=wA, in_=w1_d[0:3].rearrange("k i o -> i k o"))
        # conv1 weights (w_inh), cast f32->bf16 via gpsimd DMA
        wc1 = wpool.tile([H, 25, H], BF16, tag="w5", name="wc1")
        nc.gpsimd.dma_start(out=wc1, in_=w5_d[0].rearrange("t i o -> i t o"))

        def clock_gate(psum_c, bias_col, sig):
            """sig *= cos(psum_c + bias)^2  (in place)"""
            clk = sc.tile([H, CHW], F32, tag="clk", name="clk")
            nc.vector.tensor_scalar(out=clk, in0=psum_c,
                                    scalar1=par[:, bias_col:bias_col + 1],
                                    scalar2=None, op0=OP.add)
            nc.vector.add_range_wrap(clk, clk, shift=0.0, bound=PI,
                                     period=2 * PI)
            nc.vector.add_range_wrap(clk, clk, shift=0.0, bound=PI,
                                     period=2 * PI)
            nc.scalar.activation(clk, clk, AF.Sin)
            nc.vector.tensor_tensor(clk, clk, clk, OP.mult)
            nc.vector.tensor_tensor(sig, sig, clk, OP.mult)
            return sig

        # ---------------- Loop A: att_gate, g = exc*att -> pad ----------
        for j in range(NCH):
            c0 = j * CHW
            sx = xs.tile([H, CHW], F32, tag="sx", name="sx")
            nc.sync.dma_start(out=sx, in_=x_d[:, c0:c0 + CHW])
            ps = pp.tile([H, CHW], F32, tag="p0", name="ps")
            nc.tensor.matmul(ps, wA[:, 0, :], sx, start=True, stop=False)
            nc.tensor.matmul(ps, wA[:, 1, :], exc_sb[:, c0:c0 + CHW],
                             start=False, stop=True)
            pc = pp.tile([H, CHW], F32, tag="p1", name="pc")
            nc.tensor.matmul(pc, wA[:, 2, :], exc_sb[:, c0:c0 + CHW],
                             start=True, stop=True)
            sig = sc.tile([H, CHW], F32, tag="sig", name="sig")
            nc.scalar.activation(sig, ps, AF.Sigmoid,
                                 bias=par[:, C_BATT:C_BATT + 1], scale=1.0)
            clock_gate(pc, C_BCA, sig)
            r0 = 2 + j * CHR
            nc.vector.tensor_tensor(
                pad[:, r0:r0 + CHR, 2:2 + S],
                exc_sb[:, c0:c0 + CHW].rearrange("p (r c) -> p r c", r=CHR),
                sig.rearrange("p (r c) -> p r c", r=CHR),
                OP.mult)

        # ---------------- conv (shared emitter) -------------------------
        def conv5(wtile, sums, sumsq):
            for grp in range(NCH // CGRP):
                pts = [pp.tile([H, CHW], F32, tag=f"p{i}", name=f"pcv{i}")
                       for i in range(CGRP)]
                for t in range(25):
                    dy, dx = t // 5, t % 5
                    for i in range(CGRP):
                        y0 = (grp * CGRP + i) * CHR
                        rhs = pad[:, y0 + dy:y0 + dy + CHR, dx:dx + S]
                        nc.tensor.matmul(pts[i], wtile[:, t, :], rhs,
                                         start=(t == 0), stop=(t == 24))
                for i in range(CGRP):
                    ch = grp * CGRP + i
                    c0 = ch * CHW
                    nc.scalar.activation(intx[:, c0:c0 + CHW], pts[i],
                                         AF.Copy,
                                         accum_out=sums[:, ch:ch + 1])
                    sq = sc.tile([H, CHW], F32, tag="sq", name="sq")
                    nc.scalar.activation(sq, pts[i], AF.Square,
                                         accum_out=sumsq[:, ch:ch + 1])

        def bn_coeffs(sums, sumsq, wcol, bcol, tagp):
            """AllReduce stats across cores; return (scale, bias) [H,1]."""
            st = stp.tile([H, 2], F32, name=f"st{tagp}")
            nc.vector.reduce_sum(out=st[:, 0:1], in_=sums,
                                 axis=mybir.AxisListType.X)
            nc.vector.reduce_sum(out=st[:, 1:2], in_=sumsq,
                                 axis=mybir.AxisListType.X)
            cin = dp.tile([H, 2], F32, tag="cin", name=f"cin{tagp}")
            cout = dp.tile([H, 2], F32, tag="cout", name=f"cout{tagp}")
            nc.gpsimd.dma_start(out=cin, in_=st)
            nc.gpsimd.collective_compute(
                "AllReduce", OP.add,
                replica_groups=[list(range(NCORES))],
                ins=[cin.opt()], outs=[cout.opt()])
            stg = stp.tile([H, 2], F32, name=f"stg{tagp}")
            nc.gpsimd.dma_start(out=stg, in_=cout)
            m = stp.tile([H, 1], F32, name=f"m{tagp}")
            nc.vector.tensor_scalar(out=m, in0=stg[:, 0:1], scalar1=1.0 / NTOT,
                                    scalar2=None, op0=OP.mult)
            v = stp.tile([H, 1], F32, name=f"v{tagp}")
            # v = E[x^2] - m^2 = sumsq/N - m*m
            nc.vector.tensor_scalar(out=v, in0=stg[:, 1:2], scalar1=1.0 / NTOT,
                                    scalar2=None, op0=OP.mult)
            m2 = stp.tile([H, 1], F32, name=f"m2{tagp}")
            nc.vector.tensor_tensor(m2, m, m, OP.mult)
            nc.vector.tensor_tensor(v, v, m2, OP.subtract)
            # sd = sqrt(v + eps); rstd = 1/sd
            nc.scalar.activation(v, v, AF.Sqrt, bias=eps_sb, scale=1.0)
            rstd = stp.tile([H, 1], F32, name=f"rs{tagp}")
            nc.vector.reciprocal(rstd, v)
            scl = stp.tile([H, 1], F32, name=f"scl{tagp}")
            nc.vector.tensor_tensor(scl, rstd, par[:, wcol:wcol + 1], OP.mult)
            bia = stp.tile([H, 1], F32, name=f"bia{tagp}")
            nc.vector.tensor_tensor(bia, m, scl, OP.mult)
            nc.vector.tensor_tensor(bia, par[:, bcol:bcol + 1], bia,
                                    OP.subtract)
            return scl, bia

        sums0 = stp.tile([H, NCH], F32, name="sums0")
        sumsq0 = stp.tile([H, NCH], F32, name="sumsq0")
        conv5(wc1, sums0, sumsq0)
        bn0_s, bn0_b = bn_coeffs(sums0, sumsq0, C_BN0W, C_BN0B, "a")

        # stage-B weights + conv2 weights (prefetch during conv1)
        wB = wpool.tile([H, 3, H], F32, tag="w1", name="wB")
        nc.sync.dma_start(out=wB, in_=w1_d[3:6].rearrange("k i o -> i k o"))
        wc2 = wpool.tile([H, 25, H], BF16, tag="w5", name="wc2")
        nc.gpsimd.dma_start(out=wc2, in_=w5_d[1].rearrange("t i o -> i t o"))

        # -------- Loop B: inh gate + inhibition_hat + blend -------------
        for j in range(NCH):
            c0 = j * CHW
            inh_c = inh_sb[:, c0:c0 + CHW]
            sx = xs.tile([H, CHW], F32, tag="sx", name="sxb")
            nc.sync.dma_start(out=sx, in_=x_d[:, c0:c0 + CHW])
            ps = pp.tile([H, CHW], F32, tag="p0", name="psb")
            nc.tensor.matmul(ps, wB[:, 0, :], sx, start=True, stop=False)
            nc.tensor.matmul(ps, wB[:, 1, :], inh_c, start=False, stop=True)
            pc = pp.tile([H, CHW], F32, tag="p1", name="pcb")
            nc.tensor.matmul(pc, wB[:, 2, :], inh_c, start=True, stop=True)
            sig = sc.tile([H, CHW], F32, tag="sig", name="sigb")
            nc.scalar.activation(sig, ps, AF.Sigmoid,
                                 bias=par[:, C_BINH:C_BINH + 1], scale=1.0)
            clock_gate(pc, C_BCI, sig)

            # inhibition_hat = tanh(x - tanh(bn0(intx) * (alpha*inh + mu)))
            t = sc.tile([H, CHW], F32, tag="t", name="tb")
            nc.scalar.activation(t, intx[:, c0:c0 + CHW], AF.Identity,
                                 bias=bn0_b, scale=bn0_s)
            am = sc.tile([H, CHW], F32, tag="am", name="am")
            nc.vector.tensor_scalar(out=am, in0=inh_c,
                                    scalar1=par[:, C_ALPHA:C_ALPHA + 1],
                                    scalar2=par[:, C_MU:C_MU + 1],
                                    op0=OP.mult, op1=OP.add)
            nc.vector.tensor_tensor(t, t, am, OP.mult)
            nc.scalar.activation(t, t, AF.Tanh)
            nc.vector.tensor_tensor(t, sx, t, OP.subtract)
            nc.scalar.activation(t, t, AF.Tanh)
            # blend: inh_new = inh + sig*(ihat - inh)   (in place into inh_sb)
            nc.vector.tensor_tensor(t, t, inh_c, OP.subtract)
            nc.vector.tensor_tensor(t, t, sig, OP.mult)
            nc.vector.tensor_tensor(inh_c, inh_c, t, OP.add)
            # write to pad (bf16) for conv2 + out DMA
            r0 = 2 + j * CHR
            nc.vector.tensor_copy(
                out=pad[:, r0:r0 + CHR, 2:2 + S],
                in_=inh_c.rearrange("p (r c) -> p r c", r=CHR))
            nc.sync.dma_start(out=oinh_d[:, c0:c0 + CHW], in_=inh_c)

        sums1 = stp.tile([H, NCH], F32, name="sums1")
        sumsq1 = stp.tile([H, NCH], F32, name="sumsq1")
        conv5(wc2, sums1, sumsq1)
        bn1_s, bn1_b = bn_coeffs(sums1, sumsq1, C_BN1W, C_BN1B, "b")

        wC = wpool.tile([H, 3, H], F32, tag="w1", name="wC")
        nc.sync.dma_start(out=wC, in_=w1_d[6:9].rearrange("k i o -> i k o"))

        # -------- Loop C: exc gate + excitation_hat + blend -------------
        for j in range(NCH):
            c0 = j * CHW
            inh_c = inh_sb[:, c0:c0 + CHW]   # inhibition_new
            exc_c = exc_sb[:, c0:c0 + CHW]
            ps = pp.tile([H, CHW], F32, tag="p0", name="psc")
            nc.tensor.matmul(ps, wC[:, 0, :], inh_c, start=True, stop=False)
            nc.tensor.matmul(ps, wC[:, 1, :], exc_c, start=False, stop=True)
            pc = pp.tile([H, CHW], F32, tag="p1", name="pcc")
            nc.tensor.matmul(pc, wC[:, 2, :], exc_c, start=True, stop=True)
            sig = sc.tile([H, CHW], F32, tag="sig", name="sigc")
            nc.scalar.activation(sig, ps, AF.Sigmoid,
                                 bias=par[:, C_BEXC:C_BEXC + 1], scale=1.0)
            clock_gate(pc, C_BCE, sig)

            # ehat = tanh(kappa*r + gamma*bn + w_gain*r*bn),  r = inh_new
            t = sc.tile([H, CHW], F32, tag="t", name="tc")
            nc.scalar.activation(t, intx[:, c0:c0 + CHW], AF.Identity,
                                 bias=bn1_b, scale=bn1_s)
            p1 = sc.tile([H, CHW], F32, tag="am", name="p1c")
            nc.vector.tensor_scalar(out=p1, in0=t,
                                    scalar1=par[:, C_GAMMA:C_GAMMA + 1],
                                    scalar2=None, op0=OP.mult)
            nc.vector.scalar_tensor_tensor(out=p1, in0=inh_c,
                                           scalar=par[:, C_KAPPA:C_KAPPA + 1],
                                           in1=p1, op0=OP.mult, op1=OP.add)
            nc.vector.tensor_tensor(t, inh_c, t, OP.mult)
            nc.vector.scalar_tensor_tensor(out=t, in0=t,
                                           scalar=par[:, C_WGAIN:C_WGAIN + 1],
                                           in1=p1, op0=OP.mult, op1=OP.add)
            nc.scalar.activation(t, t, AF.Tanh)
            # blend: exc_new = exc + sig*(ehat - exc)   (in place)
            nc.vector.tensor_tensor(t, t, exc_c, OP.subtract)
            nc.vector.tensor_tensor(t, t, sig, OP.mult)
            nc.vector.tensor_tensor(exc_c, exc_c, t, OP.add)
            nc.sync.dma_start(out=oexc_d[:, c0:c0 + CHW], in_=exc_c)

    nc.compile()
    return nc


_NC_CACHE = None


def _get_program():
    global _NC_CACHE
    if _NC_CACHE is None:
        _NC_CACHE = _build_program()
    return _NC_CACHE


def _build_in_maps(input_, inhibition, excitation,
                   aw_w, aw_b, au_w, au_b, iw_w, iw_b, iu_w, iu_b,
                   ew_w, ew_b, eu_w, eu_b,
                   ac_w, ac_b, ic_w, ic_b, ec_w, ec_b,
                   w_inh, w_exc, alpha, gamma, kappa, w_gain, mu,
                   bn0_w, bn0_b, bn1_w, bn1_b, step):
    f = lambda a: np.ascontiguousarray(np.asarray(a, dtype=np.float32))
    stepf = float(np.asarray(step))

    input_, inhibition, excitation = f(input_), f(inhibition), f(excitation)

    # 1x1 weights, transposed to [I, O]; clock weights pre-scaled by step
    w1 = np.stack([
        f(aw_w).T, f(au_w).T, f(ac_w).T * stepf,
        f(iw_w).T, f(iu_w).T, f(ic_w).T * stepf,
        f(ew_w).T, f(eu_w).T, f(ec_w).T * stepf,
    ]).astype(np.float32)
    # 5x5 weights -> [25, I, O]
    w5 = np.stack([
        f(w_inh).transpose(2, 3, 1, 0).reshape(25, H, H),
        f(w_exc).transpose(2, 3, 1, 0).reshape(25, H, H),
    ]).astype(np.float32)

    chan = lambda a: f(a).reshape(H)
    par = np.zeros((H, 16), dtype=np.float32)
    par[:, 0] = chan(aw_b) + chan(au_b)
    par[:, 1] = chan(ac_b) * stepf + np.pi / 2
    par[:, 2] = chan(iw_b) + chan(iu_b)
    par[:, 3] = chan(ic_b) * stepf + np.pi / 2
    par[:, 4] = chan(ew_b) + chan(eu_b)
    par[:, 5] = chan(ec_b) * stepf + np.pi / 2
    par[:, 6] = chan(alpha)
    par[:, 7] = chan(mu)
    par[:, 8] = chan(kappa)
    par[:, 9] = chan(gamma)
    par[:, 10] = chan(w_gain)
    par[:, 11] = chan(bn0_w)
    par[:, 12] = chan(bn0_b)
    par[:, 13] = chan(bn1_w)
    par[:, 14] = chan(bn1_b)

    in_maps = []
    for b in range(NCORES):
        in_maps.append({
            "x": input_[b].reshape(H, SS),
            "inh": inhibition[b].reshape(H, SS),
            "exc": excitation[b].reshape(H, SS),
            "w1x1": w1,
            "w5": w5,
            "params": par,
        })
    return in_maps


def kernel(**inputs):
    in_maps = _build_in_maps(**inputs)
    nc = _get_program()
    res = run_bass_kernel_spmd(nc, in_maps, list(range(NCORES)))

    inh_new = np.stack([res.results[b]["out_inh"].reshape(H, S, S)
                        for b in range(NCORES)])
    exc_new = np.stack([res.results[b]["out_exc"].reshape(H, S, S)
                        for b in range(NCORES)])
    return inh_new.astype(np.float32), exc_new.astype(np.float32)


# revision 12
# speedup vs baseline: 1.1328x; 1.1328x over previous
"""ClockHConvGRUCell on 8 Trainium2 NeuronCores — data-parallel over batch.

Contract: kernel(**inputs) takes the FULL unsharded inputs (numpy), returns
(inhibition_new, excitation_new) with full shapes [8,128,96,96] f32.

Per-core layout: channels (H=128) on SBUF partitions, spatial (96*96=9216) on
the free dim.  1x1 convs are bf16 128x128 matmuls over 384-column chunks; the
5x5 convs are 25 shifted bf16 matmuls accumulated in PSUM over a zero-padded
bf16 [100x100] image.  BatchNorm batch-stats (bn_stats/bn_aggr) are
AllReduce'd across the 8 cores.

cos^2(t) is computed as sin^2(z) with z = |t + pi/2| mod 2pi - pi, which
needs only two DVE tensor_scalar ops of range reduction before the ACT Sin.
ACT ops are batched per function (sigmoid streams, then sin, then tanh) to
avoid activation-table reloads.
"""

import math
import sys

sys.path.insert(0, "/opt/trn_rl_repo")

import numpy as np

import concourse.bacc as bacc
import concourse.bass as bass
import concourse.tile as tile
from concourse import mybir
from concourse.bass_utils import run_bass_kernel_spmd

F32 = mybir.dt.float32
BF16 = mybir.dt.bfloat16
FP16 = mybir.dt.float16
AF = mybir.ActivationFunctionType
OP = mybir.AluOpType

H = 128
S = 96
SS = S * S          # 9216
W = S + 4           # padded width/height (2 halo each side)
NCORES = 8
CHR = 4             # output rows per chunk
NCH = S // CHR      # 24 chunks
CHW = CHR * S       # 384 columns per chunk
CGRP = 4            # conv chunks per weight sweep (4 psum banks)
BW = 6 * CHW        # 2304: batch width for chains (4 batches)
HLFW = SS // 2      # 4608: half-width batches for sin
CW = 4 * CHW        # 1536: chain/stream batch width (6 batches)
PI = math.pi
EPS = 1e-3
NTOT = float(NCORES * SS)

# params columns
C_BATT, C_BCA, C_BINH, C_BCI, C_BEXC, C_BCE = 0, 1, 2, 3, 4, 5
C_ALPHA, C_MU, C_KAPPA, C_GAMMA, C_WGAIN = 6, 7, 8, 9, 10
C_BN0W, C_BN0B, C_BN1W, C_BN1B = 11, 12, 13, 14


def _build_program():
    nc = bacc.Bacc("TRN2", target_bir_lowering=False, debug=False,
                   num_devices=NCORES)

    x_d = nc.dram_tensor("x", [H, SS], F32, kind="ExternalInput").ap()
    inh_d = nc.dram_tensor("inh", [H, SS], F32, kind="ExternalInput").ap()
    exc_d = nc.dram_tensor("exc", [H, SS], F32, kind="ExternalInput").ap()
    w1_d = nc.dram_tensor("w1x1", [9, H, H], F32, kind="ExternalInput").ap()
    w5_d = nc.dram_tensor("w5", [2, 25, H, H], F32, kind="ExternalInput").ap()
    par_d = nc.dram_tensor("params", [H, 16], F32, kind="ExternalInput").ap()
    oinh_d = nc.dram_tensor("out_inh", [H, SS], F32, kind="ExternalOutput").ap()
    oexc_d = nc.dram_tensor("out_exc", [H, SS], F32, kind="ExternalOutput").ap()

    from contextlib import ExitStack
    with tile.TileContext(nc) as tc, ExitStack() as ctx:
        const = ctx.enter_context(tc.tile_pool(name="const", bufs=1))
        wpool = ctx.enter_context(tc.tile_pool(name="wpool", bufs=2))
        strm = ctx.enter_context(tc.tile_pool(name="strm", bufs=2))
        chn = ctx.enter_context(tc.tile_pool(name="chn", bufs=1))
        sc = ctx.enter_context(tc.tile_pool(name="sc", bufs=3))
        stp = ctx.enter_context(tc.tile_pool(name="stp", bufs=1))
        pp = ctx.enter_context(tc.tile_pool(name="pp", bufs=1, space="PSUM"))
        dp = ctx.enter_context(tc.tile_pool(name="dp", bufs=2, space="DRAM"))

        par = const.tile([H, 16], F32, name="par")
        nc.sync.dma_start(out=par, in_=par_d)
        eps_sb = const.tile([H, 1], F32, name="eps_sb")
        nc.vector.memset(eps_sb, EPS)

        inh_sb = const.tile([H, SS], F32, name="inh_sb")
        intx = const.tile([H, SS], F32, name="intx")
        pad = const.tile([H, W, W], BF16, name="pad")
        nc.vector.memset(pad, 0.0)
        gate_b = const.tile([H, SS], BF16, name="gate_b")
        clk = const.tile([H, SS], FP16, name="clk")
        excb = const.tile([H, SS], BF16, name="excb")

        for q in range(4):
            c0 = q * (SS // 4)
            nc.sync.dma_start(out=inh_sb[:, c0:c0 + SS // 4],
                              in_=inh_d[:, c0:c0 + SS // 4])
        for q in range(4):
            c0 = q * (SS // 4)
            nc.gpsimd.dma_start(out=excb[:, c0:c0 + SS // 4],
                                in_=exc_d[:, c0:c0 + SS // 4])

        # stage-A 1x1 weights (aw, au, ac) as bf16; conv1 weights bf16
        wA = wpool.tile([H, 3, H], BF16, tag="w1", name="wA")
        nc.gpsimd.dma_start(out=wA, in_=w1_d[0:3].rearrange("k i o -> i k o"))
        wc1 = wpool.tile([H, 25, H], BF16, tag="w5", bufs=1, name="wc1")
        nc.gpsimd.dma_start(out=wc1, in_=w5_d[0].rearrange("t i o -> i t o"))

        def gate_mms(j, wt, rhs_a, rhs_b, bias_sig, bias_clk):
            """Emit the 3 gate matmuls for 384-chunk j + sigmoid + clock
            range-reduction step 1.  rhs_a/rhs_b are bf16 [H, CHW] APs."""
            c0 = j * CHW
            ps = pp.tile([H, CHW], F32, tag="ps", bufs=2, name=f"ps{j}")
            nc.tensor.matmul(ps, wt[:, 0, :], rhs_a, start=True, stop=False)
            nc.tensor.matmul(ps, wt[:, 1, :], rhs_b, start=False, stop=True)
            pc = pp.tile([H, CHW], F32, tag="pc", bufs=2, name=f"pc{j}")
            nc.tensor.matmul(pc, wt[:, 2, :], rhs_b, start=True, stop=True)
            nc.scalar.activation(gate_b[:, c0:c0 + CHW], ps, AF.Sigmoid,
                                 bias=par[:, bias_sig:bias_sig + 1], scale=1.0)
            # clk = psum + bias   (bias includes step*b + pi/2)
            nc.vector.tensor_scalar(out=clk[:, c0:c0 + CHW], in0=pc,
                                    scalar1=par[:, bias_clk:bias_clk + 1],
                                    scalar2=None, op0=OP.add)

        def clock_finish(h):
            """range-reduce clk into [-pi, pi], sin -> clk, over half h."""
            v = clk[:, h * HLFW:(h + 1) * HLFW]
            nc.vector.add_range_wrap(v, v, shift=0.0, bound=PI, period=2 * PI)
            nc.vector.add_range_wrap(v, v, shift=0.0, bound=PI, period=2 * PI)
            nc.scalar.activation(v, v, AF.Sin)

        def gate_finish(s):
            """gate_b *= sin^2 over sixth s."""
            c0 = s * BW
            c2 = sc.tile([H, BW], BF16, tag="c2", bufs=1, name=f"c2_{s}")
            nc.vector.tensor_tensor(c2, clk[:, c0:c0 + BW],
                                    clk[:, c0:c0 + BW], OP.mult)
            nc.vector.tensor_tensor(gate_b[:, c0:c0 + BW],
                                    gate_b[:, c0:c0 + BW], c2, OP.mult)

        # ---------------- Loop A: att gate, g = exc*gate -> pad ----------
        # processed in halves so conv1 can start while half 2 is finishing
        for h in range(2):
            xq = [None, None]
            for qq in range(2):
                q = h * 2 + qq
                xq[qq] = strm.tile([H, BW], BF16, tag="xbf",
                                   name=f"xa{q}")
                nc.gpsimd.dma_start(out=xq[qq],
                                    in_=x_d[:, q * BW:(q + 1) * BW])
            for jj in range(12):
                j = h * 12 + jj
                c0 = j * CHW
                xa = xq[jj // 6][:, (jj % 6) * CHW:(jj % 6 + 1) * CHW]
                gate_mms(j, wA, xa, excb[:, c0:c0 + CHW], C_BATT, C_BCA)
            clock_finish(h)
            for ss_ in range(2):
                s = h * 2 + ss_
                gate_finish(s)
                for jj in range(6):
                    j = s * 6 + jj
                    c0 = j * CHW
                    r0 = 2 + j * CHR
                    nc.vector.tensor_tensor(
                        pad[:, r0:r0 + CHR, 2:2 + S],
                        excb[:, c0:c0 + CHW].rearrange("p (r c) -> p r c",
                                                       r=CHR),
                        gate_b[:, c0:c0 + CHW].rearrange("p (r c) -> p r c",
                                                         r=CHR),
                        OP.mult)

        # ---------------- conv (shared emitter) -------------------------
        def conv5(wtile, stats_t):
            for grp in range(NCH // CGRP):
                pts = [pp.tile([H, CHW], F32, tag=f"p{i}", bufs=1,
                               name=f"pcv{grp}_{i}")
                       for i in range(CGRP)]
                for t in range(25):
                    dy, dx = t // 5, t % 5
                    for i in range(CGRP):
                        y0 = (grp * CGRP + i) * CHR
                        rhs = pad[:, y0 + dy:y0 + dy + CHR, dx:dx + S]
                        nc.tensor.matmul(pts[i], wtile[:, t, :], rhs,
                                         start=(t == 0), stop=(t == 24))
                for i in range(CGRP):
                    ch = grp * CGRP + i
                    c0 = ch * CHW
                    nc.vector.tensor_copy(out=intx[:, c0:c0 + CHW],
                                          in_=pts[i])
                    nc.vector.bn_stats(out=stats_t[:, ch, :], in_=pts[i])

        def bn_coeffs(stats_t, wcol, bcol, tagp):
            """bn_aggr -> (sum, sumsq) -> AllReduce -> scale/bias [H,1]."""
            mv = stp.tile([H, 2], F32, name=f"mv{tagp}")
            nc.vector.bn_aggr(out=mv, in_=stats_t)
            m2 = stp.tile([H, 1], F32, name=f"m2{tagp}")
            nc.vector.tensor_tensor(m2, mv[:, 0:1], mv[:, 0:1], OP.mult)
            st = stp.tile([H, 2], F32, name=f"st{tagp}")
            nc.vector.tensor_scalar(out=st[:, 0:1], in0=mv[:, 0:1],
                                    scalar1=float(SS), scalar2=None,
                                    op0=OP.mult)
            nc.vector.tensor_scalar(out=st[:, 1:2], in0=mv[:, 1:2],
                                    scalar1=m2, scalar2=float(SS),
                                    op0=OP.add, op1=OP.mult)
            cin = dp.tile([H, 2], F32, tag="cin", name=f"cin{tagp}")
            cout = dp.tile([H, 2], F32, tag="cout", name=f"cout{tagp}")
            nc.gpsimd.dma_start(out=cin, in_=st)
            nc.gpsimd.collective_compute(
                "AllReduce", OP.add,
                replica_groups=[list(range(NCORES))],
                ins=[cin.opt()], outs=[cout.opt()])
            stg = stp.tile([H, 2], F32, name=f"stg{tagp}")
            nc.gpsimd.dma_start(out=stg, in_=cout)
            m = stp.tile([H, 1], F32, name=f"m{tagp}")
            nc.vector.tensor_scalar(out=m, in0=stg[:, 0:1], scalar1=1.0 / NTOT,
                                    scalar2=None, op0=OP.mult)
            mm2 = stp.tile([H, 1], F32, name=f"mm2{tagp}")
            nc.vector.tensor_tensor(mm2, m, m, OP.mult)
            v = stp.tile([H, 1], F32, name=f"v{tagp}")
            # var = sumsq/N - m^2
            nc.vector.tensor_scalar(out=v, in0=stg[:, 1:2], scalar1=1.0 / NTOT,
                                    scalar2=mm2, op0=OP.mult, op1=OP.subtract)
            nc.scalar.activation(v, v, AF.Sqrt, bias=eps_sb, scale=1.0)
            rstd = stp.tile([H, 1], F32, name=f"rs{tagp}")
            nc.vector.reciprocal(rstd, v)
            scl = stp.tile([H, 1], F32, name=f"scl{tagp}")
            nc.vector.tensor_tensor(scl, rstd, par[:, wcol:wcol + 1], OP.mult)
            bia = stp.tile([H, 1], F32, name=f"bia{tagp}")
            nc.vector.tensor_tensor(bia, m, scl, OP.mult)
            nc.vector.tensor_tensor(bia, par[:, bcol:bcol + 1], bia,
                                    OP.subtract)
            return scl, bia

        # conv1 + prefetch next weights
        stats0 = stp.tile([H, NCH, 6], F32, name="stats0")
        conv5(wc1, stats0)
        wB = wpool.tile([H, 3, H], BF16, tag="w1", name="wB")
        nc.gpsimd.dma_start(out=wB, in_=w1_d[3:6].rearrange("k i o -> i k o"))
        wc2 = wpool.tile([H, 25, H], BF16, tag="w5", bufs=1, name="wc2")
        nc.gpsimd.dma_start(out=wc2, in_=w5_d[1].rearrange("t i o -> i t o"))

        # loop-B gate matmuls (independent of BN0) — fill the AllReduce gap
        for q in range(4):
            xbq = strm.tile([H, BW], BF16, tag="xbf", name=f"xb{q}")
            nc.gpsimd.dma_start(out=xbq,
                                in_=x_d[:, q * BW:(q + 1) * BW])
            ibq = strm.tile([H, BW], BF16, tag="ibf", bufs=1, name=f"ib{q}")
            nc.vector.tensor_copy(out=ibq,
                                  in_=inh_sb[:, q * BW:(q + 1) * BW])
            for jj in range(6):
                j = q * 6 + jj
                sl = slice(jj * CHW, (jj + 1) * CHW)
                gate_mms(j, wB, xbq[:, sl], ibq[:, sl], C_BINH, C_BCI)

        bn0_s, bn0_b = bn_coeffs(stats0, C_BN0W, C_BN0B, "a")
        for h in range(2):
            clock_finish(h)
        for s in range(4):
            gate_finish(s)

        # -------- chain B: inhibition_hat + blend, per batch ------------
        for s in range(6):
            c0 = s * CW
            sl = slice(c0, c0 + CW)
            sx = strm.tile([H, CW], F32, tag="strm", name=f"sx{s}")
            nc.sync.dma_start(out=sx, in_=x_d[:, sl])
            am = chn.tile([H, CW], F32, tag="am", name=f"am{s}")
            nc.vector.tensor_scalar(out=am, in0=inh_sb[:, sl],
                                    scalar1=par[:, C_ALPHA:C_ALPHA + 1],
                                    scalar2=par[:, C_MU:C_MU + 1],
                                    op0=OP.mult, op1=OP.add)
            tt = chn.tile([H, CW], F32, tag="tt", name=f"tt{s}")
            nc.vector.tensor_scalar(out=tt, in0=intx[:, sl],
                                    scalar1=bn0_s, scalar2=bn0_b,
                                    op0=OP.mult, op1=OP.add)
            nc.vector.tensor_tensor(tt, tt, am, OP.mult)
            nc.scalar.activation(tt, tt, AF.Tanh)
            nc.vector.tensor_tensor(tt, sx, tt, OP.subtract)
            nc.scalar.activation(tt, tt, AF.Tanh)
            # blend into inh_sb (in place): inh += gate*(ihat - inh)
            nc.vector.tensor_tensor(tt, tt, inh_sb[:, sl], OP.subtract)
            nc.vector.tensor_tensor(tt, tt, gate_b[:, sl], OP.mult)
            nc.vector.tensor_tensor(inh_sb[:, sl], inh_sb[:, sl], tt, OP.add)
            nc.sync.dma_start(out=oinh_d[:, sl], in_=inh_sb[:, sl])
            for jj in range(4):
                j = s * 4 + jj
                cc = j * CHW
                r0 = 2 + j * CHR
                nc.vector.tensor_copy(
                    out=pad[:, r0:r0 + CHR, 2:2 + S],
                    in_=inh_sb[:, cc:cc + CHW].rearrange("p (r c) -> p r c",
                                                         r=CHR))

        # -------- conv2 + loop-C gates --------------------------------
        stats1 = stp.tile([H, NCH, 6], F32, name="stats1")
        conv5(wc2, stats1)
        wC = wpool.tile([H, 3, H], BF16, tag="w1", name="wC")
        nc.gpsimd.dma_start(out=wC, in_=w1_d[6:9].rearrange("k i o -> i k o"))
        for q in range(4):
            ibq = strm.tile([H, BW], BF16, tag="ibf", bufs=1, name=f"ic{q}")
            nc.vector.tensor_copy(out=ibq,
                                  in_=inh_sb[:, q * BW:(q + 1) * BW])
            for jj in range(6):
                j = q * 6 + jj
                sl = slice(jj * CHW, (jj + 1) * CHW)
                c0 = j * CHW
                gate_mms(j, wC, ibq[:, sl], excb[:, c0:c0 + CHW],
                         C_BEXC, C_BCE)

        bn1_s, bn1_b = bn_coeffs(stats1, C_BN1W, C_BN1B, "b")
        for h in range(2):
            clock_finish(h)
        for s in range(4):
            gate_finish(s)

        # -------- chain C: excitation_hat + blend, per batch ------------
        for s in range(6):
            c0 = s * CW
            sl = slice(c0, c0 + CW)
            ex = strm.tile([H, CW], F32, tag="strm", name=f"ex{s}")
            nc.sync.dma_start(out=ex, in_=exc_d[:, sl])
            tt = chn.tile([H, CW], F32, tag="tt", name=f"tc{s}")
            nc.vector.tensor_scalar(out=tt, in0=intx[:, sl],
                                    scalar1=bn1_s, scalar2=bn1_b,
                                    op0=OP.mult, op1=OP.add)
            am = chn.tile([H, CW], F32, tag="am", name=f"ac{s}")
            # am = kappa*r + gamma*t
            nc.vector.tensor_scalar(out=am, in0=tt,
                                    scalar1=par[:, C_GAMMA:C_GAMMA + 1],
                                    scalar2=None, op0=OP.mult)
            nc.vector.scalar_tensor_tensor(out=am, in0=inh_sb[:, sl],
                                           scalar=par[:, C_KAPPA:C_KAPPA + 1],
                                           in1=am, op0=OP.mult, op1=OP.add)
            nc.vector.tensor_tensor(tt, inh_sb[:, sl], tt, OP.mult)
            nc.vector.scalar_tensor_tensor(out=tt, in0=tt,
                                           scalar=par[:, C_WGAIN:C_WGAIN + 1],
                                           in1=am, op0=OP.mult, op1=OP.add)
            nc.scalar.activation(tt, tt, AF.Tanh)
            # blend: out = exc + gate*(ehat - exc)
            nc.vector.tensor_tensor(tt, tt, ex, OP.subtract)
            nc.vector.tensor_tensor(tt, tt, gate_b[:, sl], OP.mult)
            nc.vector.tensor_tensor(ex, ex, tt, OP.add)
            nc.sync.dma_start(out=oexc_d[:, sl], in_=ex)

    nc.compile()
    return nc


_NC_CACHE = None


def _get_program():
    global _NC_CACHE
    if _NC_CACHE is None:
        _NC_CACHE = _build_program()
    return _NC_CACHE


def _build_in_maps(input_, inhibition, excitation,
                   aw_w, aw_b, au_w, au_b, iw_w, iw_b, iu_w, iu_b,
                   ew_w, ew_b, eu_w, eu_b,
                   ac_w, ac_b, ic_w, ic_b, ec_w, ec_b,
                   w_inh, w_exc, alpha, gamma, kappa, w_gain, mu,
                   bn0_w, bn0_b, bn1_w, bn1_b, step):
    f = lambda a: np.ascontiguousarray(np.asarray(a, dtype=np.float32))
    stepf = float(np.asarray(step))

    input_, inhibition, excitation = f(input_), f(inhibition), f(excitation)

    # 1x1 weights, transposed to [I, O]; clock weights pre-scaled by step
    w1 = np.stack([
        f(aw_w).T, f(au_w).T, f(ac_w).T * stepf,
        f(iw_w).T, f(iu_w).T, f(ic_w).T * stepf,
        f(ew_w).T, f(eu_w).T, f(ec_w).T * stepf,
    ]).astype(np.float32)
    # 5x5 weights -> [25, I, O]
    w5 = np.stack([
        f(w_inh).transpose(2, 3, 1, 0).reshape(25, H, H),
        f(w_exc).transpose(2, 3, 1, 0).reshape(25, H, H),
    ]).astype(np.float32)

    chan = lambda a: f(a).reshape(H)
    par = np.zeros((H, 16), dtype=np.float32)
    par[:, C_BATT] = chan(aw_b) + chan(au_b)
    par[:, C_BCA] = chan(ac_b) * stepf + np.pi / 2
    par[:, C_BINH] = chan(iw_b) + chan(iu_b)
    par[:, C_BCI] = chan(ic_b) * stepf + np.pi / 2
    par[:, C_BEXC] = chan(ew_b) + chan(eu_b)
    par[:, C_BCE] = chan(ec_b) * stepf + np.pi / 2
    par[:, C_ALPHA] = chan(alpha)
    par[:, C_MU] = chan(mu)
    par[:, C_KAPPA] = chan(kappa)
    par[:, C_GAMMA] = chan(gamma)
    par[:, C_WGAIN] = chan(w_gain)
    par[:, C_BN0W] = chan(bn0_w)
    par[:, C_BN0B] = chan(bn0_b)
    par[:, C_BN1W] = chan(bn1_w)
    par[:, C_BN1B] = chan(bn1_b)

    in_maps = []
    for b in range(NCORES):
        in_maps.append({
            "x": input_[b].reshape(H, SS),
            "inh": inhibition[b].reshape(H, SS),
            "exc": excitation[b].reshape(H, SS),
            "w1x1": w1,
            "w5": w5,
            "params": par,
        })
    return in_maps


def kernel(**inputs):
    in_maps = _build_in_maps(**inputs)
    nc = _get_program()
    res = run_bass_kernel_spmd(nc, in_maps, list(range(NCORES)))

    inh_new = np.stack([res.results[b]["out_inh"].reshape(H, S, S)
                        for b in range(NCORES)])
    exc_new = np.stack([res.results[b]["out_exc"].reshape(H, S, S)
                        for b in range(NCORES)])
    return inh_new.astype(np.float32), exc_new.astype(np.float32)


# revision 13
# speedup vs baseline: 1.1404x; 1.0067x over previous
"""ClockHConvGRUCell on 8 Trainium2 NeuronCores — data-parallel over batch.

Contract: kernel(**inputs) takes the FULL unsharded inputs (numpy), returns
(inhibition_new, excitation_new) with full shapes [8,128,96,96] f32.

Per-core layout: channels (H=128) on SBUF partitions, spatial (96*96=9216) on
the free dim.  1x1 convs are bf16 128x128 matmuls over 384-column chunks; the
5x5 convs are 25 shifted bf16 matmuls accumulated in PSUM over a zero-padded
bf16 [100x100] image.  BatchNorm batch-stats (bn_stats/bn_aggr) are
AllReduce'd across the 8 cores.

cos^2(t) is computed as sin^2(z) with z = |t + pi/2| mod 2pi - pi, which
needs only two DVE tensor_scalar ops of range reduction before the ACT Sin.
ACT ops are batched per function (sigmoid streams, then sin, then tanh) to
avoid activation-table reloads.
"""

import math
import sys

sys.path.insert(0, "/opt/trn_rl_repo")

import numpy as np

import concourse.bacc as bacc
import concourse.bass as bass
import concourse.tile as tile
from concourse import mybir
from concourse.bass_utils import run_bass_kernel_spmd

F32 = mybir.dt.float32
BF16 = mybir.dt.bfloat16
FP16 = mybir.dt.float16
AF = mybir.ActivationFunctionType
OP = mybir.AluOpType

H = 128
S = 96
SS = S * S          # 9216
W = S + 4           # padded width/height (2 halo each side)
NCORES = 8
CHR = 4             # output rows per chunk
NCH = S // CHR      # 24 chunks
CHW = CHR * S       # 384 columns per chunk
CGRP = 4            # conv chunks per weight sweep (4 psum banks)
BW = 6 * CHW        # 2304: batch width for chains (4 batches)
HLFW = SS // 2      # 4608: half-width batches for sin
CW = 4 * CHW        # 1536: chain/stream batch width (6 batches)
PI = math.pi
EPS = 1e-3
NTOT = float(NCORES * SS)

# params columns
C_BATT, C_BCA, C_BINH, C_BCI, C_BEXC, C_BCE = 0, 1, 2, 3, 4, 5
C_ALPHA, C_MU, C_KAPPA, C_GAMMA, C_WGAIN = 6, 7, 8, 9, 10
C_BN0W, C_BN0B, C_BN1W, C_BN1B = 11, 12, 13, 14


def _build_program():
    nc = bacc.Bacc("TRN2", target_bir_lowering=False, debug=False,
                   num_devices=NCORES)

    x_d = nc.dram_tensor("x", [H, SS], F32, kind="ExternalInput").ap()
    inh_d = nc.dram_tensor("inh", [H, SS], F32, kind="ExternalInput").ap()
    exc_d = nc.dram_tensor("exc", [H, SS], F32, kind="ExternalInput").ap()
    w1_d = nc.dram_tensor("w1x1", [9, H, H], F32, kind="ExternalInput").ap()
    w5_d = nc.dram_tensor("w5", [2, 25, H, H], F32, kind="ExternalInput").ap()
    par_d = nc.dram_tensor("params", [H, 16], F32, kind="ExternalInput").ap()
    oinh_d = nc.dram_tensor("out_inh", [H, SS], F32, kind="ExternalOutput").ap()
    oexc_d = nc.dram_tensor("out_exc", [H, SS], F32, kind="ExternalOutput").ap()

    from contextlib import ExitStack
    with tile.TileContext(nc) as tc, ExitStack() as ctx:
        const = ctx.enter_context(tc.tile_pool(name="const", bufs=1))
        wpool = ctx.enter_context(tc.tile_pool(name="wpool", bufs=2))
        strm = ctx.enter_context(tc.tile_pool(name="strm", bufs=2))
        chn = ctx.enter_context(tc.tile_pool(name="chn", bufs=1))
        sc = ctx.enter_context(tc.tile_pool(name="sc", bufs=3))
        stp = ctx.enter_context(tc.tile_pool(name="stp", bufs=1))
        pp = ctx.enter_context(tc.tile_pool(name="pp", bufs=1, space="PSUM"))
        dp = ctx.enter_context(tc.tile_pool(name="dp", bufs=2, space="DRAM"))

        par = const.tile([H, 16], F32, name="par")
        nc.sync.dma_start(out=par, in_=par_d)
        eps_sb = const.tile([H, 1], F32, name="eps_sb")
        nc.vector.memset(eps_sb, EPS)

        inh_sb = const.tile([H, SS], F32, name="inh_sb")
        intx = const.tile([H, SS], F32, name="intx")
        pad = const.tile([H, W, W], FP16, name="pad")
        nc.gpsimd.memset(pad, 0.0)
        gate_b = const.tile([H, SS], FP16, name="gate_b")
        clk = const.tile([H, SS], FP16, name="clk")
        excb = const.tile([H, SS], FP16, name="excb")

        for q in range(4):
            c0 = q * (SS // 4)
            nc.sync.dma_start(out=inh_sb[:, c0:c0 + SS // 4],
                              in_=inh_d[:, c0:c0 + SS // 4])
        for q in range(4):
            c0 = q * (SS // 4)
            nc.gpsimd.dma_start(out=excb[:, c0:c0 + SS // 4],
                                in_=exc_d[:, c0:c0 + SS // 4])

        # stage-A 1x1 weights (aw, au, ac) as bf16; conv1 weights bf16
        wA = wpool.tile([H, 3, H], FP16, tag="w1", name="wA")
        nc.gpsimd.dma_start(out=wA, in_=w1_d[0:3].rearrange("k i o -> i k o"))
        wc1 = wpool.tile([H, 25, H], FP16, tag="w5", bufs=1, name="wc1")
        nc.gpsimd.dma_start(out=wc1, in_=w5_d[0].rearrange("t i o -> i t o"))

        def gate_mms(j, wt, rhs_a, rhs_b, bias_sig, bias_clk):
            """Emit the 3 gate matmuls for 384-chunk j + sigmoid + clock
            range-reduction step 1.  rhs_a/rhs_b are bf16 [H, CHW] APs."""
            c0 = j * CHW
            ps = pp.tile([H, CHW], F32, tag="ps", bufs=2, name=f"ps{j}")
            nc.tensor.matmul(ps, wt[:, 0, :], rhs_a, start=True, stop=False)
            nc.tensor.matmul(ps, wt[:, 1, :], rhs_b, start=False, stop=True)
            pc = pp.tile([H, CHW], F32, tag="pc", bufs=2, name=f"pc{j}")
            nc.tensor.matmul(pc, wt[:, 2, :], rhs_b, start=True, stop=True)
            nc.scalar.activation(gate_b[:, c0:c0 + CHW], ps, AF.Sigmoid,
                                 bias=par[:, bias_sig:bias_sig + 1], scale=1.0)
            # clk = psum + bias   (bias includes step*b + pi/2)
            nc.vector.tensor_scalar(out=clk[:, c0:c0 + CHW], in0=pc,
                                    scalar1=par[:, bias_clk:bias_clk + 1],
                                    scalar2=None, op0=OP.add)

        def clock_finish(h):
            """range-reduce clk into [-pi, pi], sin -> clk, over half h."""
            v = clk[:, h * HLFW:(h + 1) * HLFW]
            nc.vector.add_range_wrap(v, v, shift=0.0, bound=PI, period=2 * PI)
            nc.vector.add_range_wrap(v, v, shift=0.0, bound=PI, period=2 * PI)
            nc.scalar.activation(v, v, AF.Sin)

        def gate_finish(s):
            """gate_b *= sin^2 over sixth s."""
            c0 = s * BW
            c2 = sc.tile([H, BW], FP16, tag="c2", bufs=1, name=f"c2_{s}")
            nc.vector.tensor_tensor(c2, clk[:, c0:c0 + BW],
                                    clk[:, c0:c0 + BW], OP.mult)
            nc.vector.tensor_tensor(gate_b[:, c0:c0 + BW],
                                    gate_b[:, c0:c0 + BW], c2, OP.mult)

        # ---------------- Loop A: att gate, g = exc*gate -> pad ----------
        # processed in halves so conv1 can start while half 2 is finishing
        for h in range(2):
            xq = [None, None]
            for qq in range(2):
                q = h * 2 + qq
                xq[qq] = strm.tile([H, BW], FP16, tag="xbf",
                                   name=f"xa{q}")
                nc.gpsimd.dma_start(out=xq[qq],
                                    in_=x_d[:, q * BW:(q + 1) * BW])
            for jj in range(12):
                j = h * 12 + jj
                c0 = j * CHW
                xa = xq[jj // 6][:, (jj % 6) * CHW:(jj % 6 + 1) * CHW]
                gate_mms(j, wA, xa, excb[:, c0:c0 + CHW], C_BATT, C_BCA)
            clock_finish(h)
            for ss_ in range(2):
                s = h * 2 + ss_
                gate_finish(s)
                for jj in range(6):
                    j = s * 6 + jj
                    c0 = j * CHW
                    r0 = 2 + j * CHR
                    nc.vector.tensor_tensor(
                        pad[:, r0:r0 + CHR, 2:2 + S],
                        excb[:, c0:c0 + CHW].rearrange("p (r c) -> p r c",
                                                       r=CHR),
                        gate_b[:, c0:c0 + CHW].rearrange("p (r c) -> p r c",
                                                         r=CHR),
                        OP.mult)

        # ---------------- conv (shared emitter) -------------------------
        def conv5(wtile, stats_t):
            for grp in range(NCH // CGRP):
                pts = [pp.tile([H, CHW], F32, tag=f"p{i}", bufs=1,
                               name=f"pcv{grp}_{i}")
                       for i in range(CGRP)]
                for t in range(25):
                    dy, dx = t // 5, t % 5
                    for i in range(CGRP):
                        y0 = (grp * CGRP + i) * CHR
                        rhs = pad[:, y0 + dy:y0 + dy + CHR, dx:dx + S]
                        nc.tensor.matmul(pts[i], wtile[:, t, :], rhs,
                                         start=(t == 0), stop=(t == 24))
                for i in range(CGRP):
                    ch = grp * CGRP + i
                    c0 = ch * CHW
                    nc.scalar.activation(intx[:, c0:c0 + CHW], pts[i],
                                         AF.Copy)
                    nc.vector.bn_stats(out=stats_t[:, ch, :], in_=pts[i])

        def bn_coeffs(stats_t, wcol, bcol, tagp):
            """bn_aggr -> (sum, sumsq) -> AllReduce -> scale/bias [H,1]."""
            mv = stp.tile([H, 2], F32, name=f"mv{tagp}")
            nc.vector.bn_aggr(out=mv, in_=stats_t)
            m2 = stp.tile([H, 1], F32, name=f"m2{tagp}")
            nc.vector.tensor_tensor(m2, mv[:, 0:1], mv[:, 0:1], OP.mult)
            st = stp.tile([H, 2], F32, name=f"st{tagp}")
            nc.vector.tensor_scalar(out=st[:, 0:1], in0=mv[:, 0:1],
                                    scalar1=float(SS), scalar2=None,
                                    op0=OP.mult)
            nc.vector.tensor_scalar(out=st[:, 1:2], in0=mv[:, 1:2],
                                    scalar1=m2, scalar2=float(SS),
                                    op0=OP.add, op1=OP.mult)
            cin = dp.tile([H, 2], F32, tag="cin", name=f"cin{tagp}")
            cout = dp.tile([H, 2], F32, tag="cout", name=f"cout{tagp}")
            nc.gpsimd.dma_start(out=cin, in_=st)
            nc.gpsimd.collective_compute(
                "AllReduce", OP.add,
                replica_groups=[list(range(NCORES))],
                ins=[cin.opt()], outs=[cout.opt()])
            stg = stp.tile([H, 2], F32, name=f"stg{tagp}")
            nc.gpsimd.dma_start(out=stg, in_=cout)
            m = stp.tile([H, 1], F32, name=f"m{tagp}")
            nc.vector.tensor_scalar(out=m, in0=stg[:, 0:1], scalar1=1.0 / NTOT,
                                    scalar2=None, op0=OP.mult)
            mm2 = stp.tile([H, 1], F32, name=f"mm2{tagp}")
            nc.vector.tensor_tensor(mm2, m, m, OP.mult)
            v = stp.tile([H, 1], F32, name=f"v{tagp}")
            # var = sumsq/N - m^2
            nc.vector.tensor_scalar(out=v, in0=stg[:, 1:2], scalar1=1.0 / NTOT,
                                    scalar2=mm2, op0=OP.mult, op1=OP.subtract)
            nc.scalar.activation(v, v, AF.Sqrt, bias=eps_sb, scale=1.0)
            rstd = stp.tile([H, 1], F32, name=f"rs{tagp}")
            nc.vector.reciprocal(rstd, v)
            scl = stp.tile([H, 1], F32, name=f"scl{tagp}")
            nc.vector.tensor_tensor(scl, rstd, par[:, wcol:wcol + 1], OP.mult)
            bia = stp.tile([H, 1], F32, name=f"bia{tagp}")
            nc.vector.tensor_tensor(bia, m, scl, OP.mult)
            nc.vector.tensor_tensor(bia, par[:, bcol:bcol + 1], bia,
                                    OP.subtract)
            return scl, bia

        # conv1 + prefetch next weights
        stats0 = stp.tile([H, NCH, 6], F32, name="stats0")
        conv5(wc1, stats0)
        wB = wpool.tile([H, 3, H], FP16, tag="w1", name="wB")
        nc.gpsimd.dma_start(out=wB, in_=w1_d[3:6].rearrange("k i o -> i k o"))
        wc2 = wpool.tile([H, 25, H], FP16, tag="w5", bufs=1, name="wc2")
        nc.gpsimd.dma_start(out=wc2, in_=w5_d[1].rearrange("t i o -> i t o"))

        # loop-B gate matmuls (independent of BN0) — fill the AllReduce gap
        for q in range(4):
            xbq = strm.tile([H, BW], FP16, tag="xbf", name=f"xb{q}")
            nc.gpsimd.dma_start(out=xbq,
                                in_=x_d[:, q * BW:(q + 1) * BW])
            ibq = strm.tile([H, BW], FP16, tag="ibf", bufs=1, name=f"ib{q}")
            nc.gpsimd.tensor_copy(out=ibq,
                                  in_=inh_sb[:, q * BW:(q + 1) * BW])
            for jj in range(6):
                j = q * 6 + jj
                sl = slice(jj * CHW, (jj + 1) * CHW)
                gate_mms(j, wB, xbq[:, sl], ibq[:, sl], C_BINH, C_BCI)

        bn0_s, bn0_b = bn_coeffs(stats0, C_BN0W, C_BN0B, "a")
        for h in range(2):
            clock_finish(h)
        for s in range(4):
            gate_finish(s)

        # -------- chain B: inhibition_hat + blend, per batch ------------
        for s in range(6):
            c0 = s * CW
            sl = slice(c0, c0 + CW)
            sx = strm.tile([H, CW], F32, tag="strm", name=f"sx{s}")
            nc.sync.dma_start(out=sx, in_=x_d[:, sl])
            am = chn.tile([H, CW], F32, tag="am", name=f"am{s}")
            nc.vector.tensor_scalar(out=am, in0=inh_sb[:, sl],
                                    scalar1=par[:, C_ALPHA:C_ALPHA + 1],
                                    scalar2=par[:, C_MU:C_MU + 1],
                                    op0=OP.mult, op1=OP.add)
            tt = chn.tile([H, CW], F32, tag="tt", name=f"tt{s}")
            junk = sc.tile([H, 1], F32, tag="junk", bufs=2, name=f"jk{s}")
            nc.vector.affine_mul_reduce(out=tt, accum_out=junk,
                                        in0=intx[:, sl], in1=am,
                                        scale=bn0_s, bias=bn0_b)
            nc.scalar.activation(tt, tt, AF.Tanh)
            nc.vector.tensor_tensor(tt, sx, tt, OP.subtract)
            nc.scalar.activation(tt, tt, AF.Tanh)
            # blend into inh_sb (in place): inh += gate*(ihat - inh)
            nc.vector.tensor_tensor(tt, tt, inh_sb[:, sl], OP.subtract)
            nc.vector.tensor_tensor(tt, tt, gate_b[:, sl], OP.mult)
            nc.vector.tensor_tensor(inh_sb[:, sl], inh_sb[:, sl], tt, OP.add)
            nc.sync.dma_start(out=oinh_d[:, sl], in_=inh_sb[:, sl])
            for jj in range(4):
                j = s * 4 + jj
                cc = j * CHW
                r0 = 2 + j * CHR
                nc.gpsimd.tensor_copy(
                    out=pad[:, r0:r0 + CHR, 2:2 + S],
                    in_=inh_sb[:, cc:cc + CHW].rearrange("p (r c) -> p r c",
                                                         r=CHR))

        # -------- conv2 + loop-C gates --------------------------------
        stats1 = stp.tile([H, NCH, 6], F32, name="stats1")
        conv5(wc2, stats1)
        wC = wpool.tile([H, 3, H], FP16, tag="w1", name="wC")
        nc.gpsimd.dma_start(out=wC, in_=w1_d[6:9].rearrange("k i o -> i k o"))
        for q in range(4):
            ibq = strm.tile([H, BW], FP16, tag="ibf", bufs=1, name=f"ic{q}")
            nc.gpsimd.tensor_copy(out=ibq,
                                  in_=inh_sb[:, q * BW:(q + 1) * BW])
            for jj in range(6):
                j = q * 6 + jj
                sl = slice(jj * CHW, (jj + 1) * CHW)
                c0 = j * CHW
                gate_mms(j, wC, ibq[:, sl], excb[:, c0:c0 + CHW],
                         C_BEXC, C_BCE)

        bn1_s, bn1_b = bn_coeffs(stats1, C_BN1W, C_BN1B, "b")
        for h in range(2):
            clock_finish(h)
        for s in range(4):
            gate_finish(s)

        # -------- chain C: excitation_hat + blend, per batch ------------
        for s in range(6):
            c0 = s * CW
            sl = slice(c0, c0 + CW)
            ex = strm.tile([H, CW], F32, tag="strm", name=f"ex{s}")
            nc.sync.dma_start(out=ex, in_=exc_d[:, sl])
            # ehat = tanh(kappa*r + (gamma + wgain*r)*bn(t)),  r = inh_new
            am = chn.tile([H, CW], F32, tag="am", name=f"ac{s}")
            nc.vector.tensor_scalar(out=am, in0=inh_sb[:, sl],
                                    scalar1=par[:, C_WGAIN:C_WGAIN + 1],
                                    scalar2=par[:, C_GAMMA:C_GAMMA + 1],
                                    op0=OP.mult, op1=OP.add)
            tt = chn.tile([H, CW], F32, tag="tt", name=f"tc{s}")
            junk = sc.tile([H, 1], F32, tag="junk", bufs=2, name=f"jc{s}")
            nc.vector.affine_mul_reduce(out=tt, accum_out=junk,
                                        in0=intx[:, sl], in1=am,
                                        scale=bn1_s, bias=bn1_b)
            nc.vector.scalar_tensor_tensor(out=tt, in0=inh_sb[:, sl],
                                           scalar=par[:, C_KAPPA:C_KAPPA + 1],
                                           in1=tt, op0=OP.mult, op1=OP.add)
            nc.scalar.activation(tt, tt, AF.Tanh)
            # blend: out = exc + gate*(ehat - exc)
            nc.vector.tensor_tensor(tt, tt, ex, OP.subtract)
            nc.vector.tensor_tensor(tt, tt, gate_b[:, sl], OP.mult)
            nc.vector.tensor_tensor(ex, ex, tt, OP.add)
            nc.sync.dma_start(out=oexc_d[:, sl], in_=ex)

    nc.compile()
    return nc


_NC_CACHE = None


def _get_program():
    global _NC_CACHE
    if _NC_CACHE is None:
        _NC_CACHE = _build_program()
    return _NC_CACHE


def _build_in_maps(input_, inhibition, excitation,
                   aw_w, aw_b, au_w, au_b, iw_w, iw_b, iu_w, iu_b,
                   ew_w, ew_b, eu_w, eu_b,
                   ac_w, ac_b, ic_w, ic_b, ec_w, ec_b,
                   w_inh, w_exc, alpha, gamma, kappa, w_gain, mu,
                   bn0_w, bn0_b, bn1_w, bn1_b, step):
    f = lambda a: np.ascontiguousarray(np.asarray(a, dtype=np.float32))
    stepf = float(np.asarray(step))

    input_, inhibition, excitation = f(input_), f(inhibition), f(excitation)

    # 1x1 weights, transposed to [I, O]; clock weights pre-scaled by step
    w1 = np.stack([
        f(aw_w).T, f(au_w).T, f(ac_w).T * stepf,
        f(iw_w).T, f(iu_w).T, f(ic_w).T * stepf,
        f(ew_w).T, f(eu_w).T, f(ec_w).T * stepf,
    ]).astype(np.float32)
    # 5x5 weights -> [25, I, O]
    w5 = np.stack([
        f(w_inh).transpose(2, 3, 1, 0).reshape(25, H, H),
        f(w_exc).transpose(2, 3, 1, 0).reshape(25, H, H),
    ]).astype(np.float32)

    chan = lambda a: f(a).reshape(H)
    par = np.zeros((H, 16), dtype=np.float32)
    par[:, C_BATT] = chan(aw_b) + chan(au_b)
    par[:, C_BCA] = chan(ac_b) * stepf + np.pi / 2
    par[:, C_BINH] = chan(iw_b) + chan(iu_b)
    par[:, C_BCI] = chan(ic_b) * stepf + np.pi / 2
    par[:, C_BEXC] = chan(ew_b) + chan(eu_b)
    par[:, C_BCE] = chan(ec_b) * stepf + np.pi / 2
    par[:, C_ALPHA] = chan(alpha)
    par[:, C_MU] = chan(mu)
    par[:, C_KAPPA] = chan(kappa)
    par[:, C_GAMMA] = chan(gamma)
    par[:, C_WGAIN] = chan(w_gain)
    par[:, C_BN0W] = chan(bn0_w)
    par[:, C_BN0B] = chan(bn0_b)
    par[:, C_BN1W] = chan(bn1_w)
    par[:, C_BN1B] = chan(bn1_b)

    in_maps = []
    for b in range(NCORES):
        in_maps.append({
            "x": input_[b].reshape(H, SS),
            "inh": inhibition[b].reshape(H, SS),
            "exc": excitation[b].reshape(H, SS),
            "w1x1": w1,
            "w5": w5,
            "params": par,
        })
    return in_maps


def kernel(**inputs):
    in_maps = _build_in_maps(**inputs)
    nc = _get_program()
    res = run_bass_kernel_spmd(nc, in_maps, list(range(NCORES)))

    inh_new = np.stack([res.results[b]["out_inh"].reshape(H, S, S)
                        for b in range(NCORES)])
    exc_new = np.stack([res.results[b]["out_exc"].reshape(H, S, S)
                        for b in range(NCORES)])
    return inh_new.astype(np.float32), exc_new.astype(np.float32)


# revision 16
# speedup vs baseline: 1.2608x; 1.1056x over previous
"""ClockHConvGRUCell on 8 Trainium2 NeuronCores — data-parallel over batch.

Contract: kernel(**inputs) takes the FULL unsharded inputs (numpy), returns
(inhibition_new, excitation_new) with full shapes [8,128,96,96] f32.

Per-core layout: channels (H=128) on SBUF partitions, spatial (96*96=9216) on
the free dim.  Activations/weights are fp16 (host-prepared); PSUM accumulates
f32.  1x1 convs are 128x128 fp16 matmuls over 384-column chunks; the 5x5
convs are 25 shifted fp16 matmuls accumulated in PSUM over a zero-padded fp16
[100x100] image.  BatchNorm batch stats (bn_stats/bn_aggr) are AllReduce'd
across the 8 cores.

cos^2(t): the clock bias rides a K=1 matmul into the clock PSUM; a first
add_range_wrap reads PSUM (f32) and stores wrapped fp16; a second wrap +
ACT Sin + ACT Square finish sin^2(z).  ACT functions are batched per type
to avoid activation-table reloads.
"""

import math
import sys

sys.path.insert(0, "/opt/trn_rl_repo")

import numpy as np

import concourse.bacc as bacc
import concourse.bass as bass
import concourse.tile as tile
from concourse import mybir
from concourse.bass_utils import run_bass_kernel_spmd

F32 = mybir.dt.float32
FP16 = mybir.dt.float16
AF = mybir.ActivationFunctionType
OP = mybir.AluOpType

H = 128
S = 96
SS = S * S          # 9216
W = S + 4           # padded width/height (2 halo each side)
NCORES = 8
CHR = 4             # output rows per chunk
NCH = S // CHR      # 24 chunks
CHW = CHR * S       # 384 columns per chunk
CGRP = 4            # conv chunks per weight sweep (4 psum banks)
BW = 6 * CHW        # 2304: chain/batch width (4 batches)
NBAT = SS // BW     # 4
HLFW = SS // 2      # 4608: half width for sin/square
PI = math.pi
EPS = 1e-3
NTOT = float(NCORES * SS)

# params columns (f32 per-channel)
C_BATT, C_BINH, C_BEXC = 0, 1, 2
C_ALPHA, C_MU, C_KAPPA, C_GAMMA, C_WGAIN = 3, 4, 5, 6, 7
C_BN0W, C_BN0B, C_BN1W, C_BN1B = 8, 9, 10, 11


def _build_program():
    nc = bacc.Bacc("TRN2", target_bir_lowering=False, debug=False,
                   num_devices=NCORES)

    x_d = nc.dram_tensor("x16", [H, SS], FP16, kind="ExternalInput").ap()
    inh_d = nc.dram_tensor("inh16", [H, SS], FP16, kind="ExternalInput").ap()
    exc_d = nc.dram_tensor("exc16", [H, SS], FP16, kind="ExternalInput").ap()
    w1_d = nc.dram_tensor("w1x1", [9, H, H], FP16, kind="ExternalInput").ap()
    w5_d = nc.dram_tensor("w5", [2, 25, H, H], FP16,
                          kind="ExternalInput").ap()
    cb_d = nc.dram_tensor("cbiasT", [3, H], FP16, kind="ExternalInput").ap()
    par_d = nc.dram_tensor("params", [H, 16], F32, kind="ExternalInput").ap()
    oinh_d = nc.dram_tensor("out_inh", [H, SS], FP16,
                            kind="ExternalOutput").ap()
    oexc_d = nc.dram_tensor("out_exc", [H, SS], FP16,
                            kind="ExternalOutput").ap()

    from contextlib import ExitStack
    with tile.TileContext(nc) as tc, ExitStack() as ctx:
        const = ctx.enter_context(tc.tile_pool(name="const", bufs=1))
        wpool = ctx.enter_context(tc.tile_pool(name="wpool", bufs=2))
        strm = ctx.enter_context(tc.tile_pool(name="strm", bufs=2))
        chn = ctx.enter_context(tc.tile_pool(name="chn", bufs=2))
        sc = ctx.enter_context(tc.tile_pool(name="sc", bufs=2))
        stp = ctx.enter_context(tc.tile_pool(name="stp", bufs=1))
        pp = ctx.enter_context(tc.tile_pool(name="pp", bufs=1, space="PSUM"))
        dp = ctx.enter_context(tc.tile_pool(name="dp", bufs=2, space="DRAM"))

        par = const.tile([H, 16], F32, name="par")
        nc.sync.dma_start(out=par, in_=par_d)
        eps_sb = const.tile([H, 1], F32, name="eps_sb")
        nc.vector.memset(eps_sb, EPS)
        ones = const.tile([1, CHW], FP16, name="ones")
        nc.vector.memset(ones, 1.0)
        cbT = const.tile([1, 3, H], FP16, name="cbT")
        nc.sync.dma_start(out=cbT[:1, :, :],
                          in_=cb_d.rearrange("k o -> (k o)"))

        inh_sb = const.tile([H, SS], FP16, name="inh_sb")
        intx = const.tile([H, SS], FP16, name="intx")
        pad = const.tile([H, W, W], FP16, name="pad")
        nc.gpsimd.memset(pad, 0.0)
        gate_b = const.tile([H, SS], FP16, name="gate_b")
        clk = const.tile([H, SS], FP16, name="clk")
        excb = const.tile([H, SS], FP16, name="excb")

        for q in range(4):
            c0 = q * (SS // 4)
            nc.sync.dma_start(out=inh_sb[:, c0:c0 + SS // 4],
                              in_=inh_d[:, c0:c0 + SS // 4])
            nc.sync.dma_start(out=excb[:, c0:c0 + SS // 4],
                              in_=exc_d[:, c0:c0 + SS // 4])

        # all 1x1 stage weights upfront (fp16, tiny); conv weights staged
        wst = []
        for k, nm in enumerate(("wA", "wB", "wC")):
            wt = wpool.tile([H, 3, H], FP16, tag="w1", bufs=3, name=nm)
            nc.sync.dma_start(out=wt,
                              in_=w1_d[3 * k:3 * k + 3].rearrange(
                                  "k i o -> i k o"))
            wst.append(wt)
        wA, wB, wC = wst
        wc1 = wpool.tile([H, 25, H], FP16, tag="w5", name="wc1")
        nc.sync.dma_start(out=wc1, in_=w5_d[0].rearrange("t i o -> i t o"))

        def gate_mms(j, wt, st, rhs_a, rhs_b, bias_sig):
            """3 gate matmuls for 384-chunk j (+K=1 clock-bias matmul),
            sigmoid into gate_b, first range-wrap PSUM->clk (fp16)."""
            c0 = j * CHW
            ps = pp.tile([H, CHW], F32, tag="ps", bufs=2, name=f"ps{st}_{j}")
            nc.tensor.matmul(ps, wt[:, 0, :], rhs_a, start=True, stop=False)
            nc.tensor.matmul(ps, wt[:, 1, :], rhs_b, start=False, stop=True)
            pc = pp.tile([H, CHW], F32, tag="pc", bufs=2, name=f"pc{st}_{j}")
            nc.tensor.matmul(pc, wt[:, 2, :], rhs_b, start=True, stop=False)
            nc.tensor.matmul(pc, cbT[:1, st, :], ones[:1, :],
                             start=False, stop=True)
            nc.scalar.activation(gate_b[:, c0:c0 + CHW], ps, AF.Sigmoid,
                                 bias=par[:, bias_sig:bias_sig + 1],
                                 scale=1.0)
            nc.vector.add_range_wrap(clk[:, c0:c0 + CHW], pc,
                                     shift=0.0, bound=PI, period=2 * PI)

        def clock_finish(h):
            """second wrap + sin + square over half h -> clk = sin^2(z)."""
            v = clk[:, h * HLFW:(h + 1) * HLFW]
            nc.vector.add_range_wrap(v, v, shift=0.0, bound=PI,
                                     period=2 * PI)
            nc.scalar.activation(v, v, AF.Sin)
            nc.scalar.activation(v, v, AF.Square)

        def gate_finish(s):
            """gate_b *= clk (sin^2) over batch s."""
            c0 = s * BW
            nc.vector.tensor_tensor(gate_b[:, c0:c0 + BW],
                                    gate_b[:, c0:c0 + BW],
                                    clk[:, c0:c0 + BW], OP.mult)

        # ---------------- Loop A: att gate, g = exc*gate -> pad ----------
        for h in range(2):
            xq = [None, None]
            for qq in range(2):
                q = h * 2 + qq
                xq[qq] = strm.tile([H, BW], FP16, tag="xbf", name=f"xa{q}")
                nc.sync.dma_start(out=xq[qq],
                                  in_=x_d[:, q * BW:(q + 1) * BW])
            for jj in range(12):
                j = h * 12 + jj
                c0 = j * CHW
                xa = xq[jj // 6][:, (jj % 6) * CHW:(jj % 6 + 1) * CHW]
                gate_mms(j, wA, 0, xa, excb[:, c0:c0 + CHW], C_BATT)
            clock_finish(h)
            for ss_ in range(2):
                s = h * 2 + ss_
                gate_finish(s)
                for jj in range(6):
                    j = s * 6 + jj
                    c0 = j * CHW
                    r0 = 2 + j * CHR
                    nc.vector.tensor_tensor(
                        pad[:, r0:r0 + CHR, 2:2 + S],
                        excb[:, c0:c0 + CHW].rearrange("p (r c) -> p r c",
                                                       r=CHR),
                        gate_b[:, c0:c0 + CHW].rearrange("p (r c) -> p r c",
                                                         r=CHR),
                        OP.mult)

        # ---------------- conv (shared emitter) -------------------------
        def conv5(wtile, stats_t):
            for grp in range(NCH // CGRP):
                pts = [pp.tile([H, CHW], F32, tag=f"p{i}", bufs=1,
                               name=f"pcv{grp}_{i}")
                       for i in range(CGRP)]
                for t in range(25):
                    dy, dx = t // 5, t % 5
                    for i in range(CGRP):
                        y0 = (grp * CGRP + i) * CHR
                        rhs = pad[:, y0 + dy:y0 + dy + CHR, dx:dx + S]
                        nc.tensor.matmul(pts[i], wtile[:, t, :], rhs,
                                         start=(t == 0), stop=(t == 24))
                for i in range(CGRP):
                    ch = grp * CGRP + i
                    c0 = ch * CHW
                    nc.scalar.activation(intx[:, c0:c0 + CHW], pts[i],
                                         AF.Copy)
                    nc.vector.bn_stats(out=stats_t[:, ch, :], in_=pts[i])

        def bn_coeffs(stats_t, wcol, bcol, tagp):
            """bn_aggr -> (sum, sumsq) -> AllReduce -> scale/bias [H,1]."""
            mv = stp.tile([H, 2], F32, name=f"mv{tagp}")
            nc.vector.bn_aggr(out=mv, in_=stats_t)
            m2 = stp.tile([H, 1], F32, name=f"m2{tagp}")
            nc.vector.tensor_tensor(m2, mv[:, 0:1], mv[:, 0:1], OP.mult)
            st = stp.tile([H, 2], F32, name=f"st{tagp}")
            nc.vector.tensor_scalar(out=st[:, 0:1], in0=mv[:, 0:1],
                                    scalar1=float(SS), scalar2=None,
                                    op0=OP.mult)
            nc.vector.tensor_scalar(out=st[:, 1:2], in0=mv[:, 1:2],
                                    scalar1=m2, scalar2=float(SS),
                                    op0=OP.add, op1=OP.mult)
            cin = dp.tile([H, 2], F32, tag="cin", name=f"cin{tagp}")
            cout = dp.tile([H, 2], F32, tag="cout", name=f"cout{tagp}")
            nc.gpsimd.dma_start(out=cin, in_=st)
            nc.gpsimd.collective_compute(
                "AllReduce", OP.add,
                replica_groups=[list(range(NCORES))],
                ins=[cin.opt()], outs=[cout.opt()])
            stg = stp.tile([H, 2], F32, name=f"stg{tagp}")
            nc.gpsimd.dma_start(out=stg, in_=cout)
            m = stp.tile([H, 1], F32, name=f"m{tagp}")
            nc.vector.tensor_scalar(out=m, in0=stg[:, 0:1],
                                    scalar1=1.0 / NTOT, scalar2=None,
                                    op0=OP.mult)
            mm2 = stp.tile([H, 1], F32, name=f"mm2{tagp}")
            nc.vector.tensor_tensor(mm2, m, m, OP.mult)
            v = stp.tile([H, 1], F32, name=f"v{tagp}")
            nc.vector.tensor_scalar(out=v, in0=stg[:, 1:2],
                                    scalar1=1.0 / NTOT, scalar2=mm2,
                                    op0=OP.mult, op1=OP.subtract)
            nc.scalar.activation(v, v, AF.Sqrt, bias=eps_sb, scale=1.0)
            rstd = stp.tile([H, 1], F32, name=f"rs{tagp}")
            nc.vector.reciprocal(rstd, v)
            scl = stp.tile([H, 1], F32, name=f"scl{tagp}")
            nc.vector.tensor_tensor(scl, rstd, par[:, wcol:wcol + 1],
                                    OP.mult)
            bia = stp.tile([H, 1], F32, name=f"bia{tagp}")
            nc.vector.tensor_tensor(bia, m, scl, OP.mult)
            nc.vector.tensor_tensor(bia, par[:, bcol:bcol + 1], bia,
                                    OP.subtract)
            return scl, bia

        # conv1 + prefetch conv2 weights
        stats0 = stp.tile([H, NCH, 6], F32, name="stats0")
        conv5(wc1, stats0)
        wc2 = wpool.tile([H, 25, H], FP16, tag="w5", name="wc2")
        nc.sync.dma_start(out=wc2, in_=w5_d[1].rearrange("t i o -> i t o"))

        # loop-B gate matmuls (independent of BN0) — fill the AllReduce gap
        for q in range(NBAT):
            xbq = strm.tile([H, BW], FP16, tag="xbf", name=f"xb{q}")
            nc.sync.dma_start(out=xbq, in_=x_d[:, q * BW:(q + 1) * BW])
            for jj in range(6):
                j = q * 6 + jj
                sl = slice(jj * CHW, (jj + 1) * CHW)
                c0 = j * CHW
                gate_mms(j, wB, 1, xbq[:, sl], inh_sb[:, c0:c0 + CHW],
                         C_BINH)
        for h in range(2):
            clock_finish(h)
        for s in range(NBAT):
            gate_finish(s)
        bn0_s, bn0_b = bn_coeffs(stats0, C_BN0W, C_BN0B, "a")

        # -------- chain B: inhibition_hat + blend, per batch ------------
        for s in range(NBAT):
            c0 = s * BW
            sl = slice(c0, c0 + BW)
            sx = strm.tile([H, BW], FP16, tag="xbf", name=f"sx{s}")
            nc.sync.dma_start(out=sx, in_=x_d[:, sl])
            am = chn.tile([H, BW], F32, tag="am", name=f"am{s}")
            nc.vector.tensor_scalar(out=am, in0=inh_sb[:, sl],
                                    scalar1=par[:, C_ALPHA:C_ALPHA + 1],
                                    scalar2=par[:, C_MU:C_MU + 1],
                                    op0=OP.mult, op1=OP.add)
            tt = chn.tile([H, BW], F32, tag="tt", name=f"tt{s}")
            junk = sc.tile([H, 1], F32, tag="junk", name=f"jk{s}")
            nc.vector.affine_mul_reduce(out=tt, accum_out=junk,
                                        in0=intx[:, sl], in1=am,
                                        scale=bn0_s, bias=bn0_b)
            nc.scalar.activation(tt, tt, AF.Tanh)
            nc.vector.tensor_tensor(tt, sx, tt, OP.subtract)
            nc.scalar.activation(tt, tt, AF.Tanh)
            # blend into inh_sb (in place): inh += gate*(ihat - inh)
            nc.vector.tensor_tensor(tt, tt, inh_sb[:, sl], OP.subtract)
            nc.vector.tensor_tensor(tt, tt, gate_b[:, sl], OP.mult)
            nc.vector.tensor_tensor(inh_sb[:, sl], inh_sb[:, sl], tt,
                                    OP.add)
            nc.sync.dma_start(out=oinh_d[:, sl], in_=inh_sb[:, sl])
            for jj in range(6):
                j = s * 6 + jj
                cc = j * CHW
                r0 = 2 + j * CHR
                nc.vector.tensor_copy(
                    out=pad[:, r0:r0 + CHR, 2:2 + S],
                    in_=inh_sb[:, cc:cc + CHW].rearrange(
                        "p (r c) -> p r c", r=CHR))

        # -------- conv2 + loop-C gates --------------------------------
        stats1 = stp.tile([H, NCH, 6], F32, name="stats1")
        conv5(wc2, stats1)
        for q in range(NBAT):
            for jj in range(6):
                j = q * 6 + jj
                c0 = j * CHW
                gate_mms(j, wC, 2, inh_sb[:, c0:c0 + CHW],
                         excb[:, c0:c0 + CHW], C_BEXC)
        for h in range(2):
            clock_finish(h)
        for s in range(NBAT):
            gate_finish(s)
        bn1_s, bn1_b = bn_coeffs(stats1, C_BN1W, C_BN1B, "b")

        # -------- chain C: excitation_hat + blend, per batch ------------
        for s in range(NBAT):
            c0 = s * BW
            sl = slice(c0, c0 + BW)
            # ehat = tanh(kappa*r + (gamma + wgain*r)*bn(t)),  r = inh_new
            am = chn.tile([H, BW], F32, tag="am", name=f"ac{s}")
            nc.vector.tensor_scalar(out=am, in0=inh_sb[:, sl],
                                    scalar1=par[:, C_WGAIN:C_WGAIN + 1],
                                    scalar2=par[:, C_GAMMA:C_GAMMA + 1],
                                    op0=OP.mult, op1=OP.add)
            tt = chn.tile([H, BW], F32, tag="tt", name=f"tc{s}")
            junk = sc.tile([H, 1], F32, tag="junk", name=f"jc{s}")
            nc.vector.affine_mul_reduce(out=tt, accum_out=junk,
                                        in0=intx[:, sl], in1=am,
                                        scale=bn1_s, bias=bn1_b)
            nc.vector.scalar_tensor_tensor(out=tt, in0=inh_sb[:, sl],
                                           scalar=par[:, C_KAPPA:C_KAPPA + 1],
                                           in1=tt, op0=OP.mult, op1=OP.add)
            nc.scalar.activation(tt, tt, AF.Tanh)
            # blend: out = exc + gate*(ehat - exc)
            nc.vector.tensor_tensor(tt, tt, excb[:, sl], OP.subtract)
            nc.vector.tensor_tensor(tt, tt, gate_b[:, sl], OP.mult)
            ot = sc.tile([H, BW], FP16, tag="ot", name=f"ot{s}")
            nc.vector.tensor_tensor(ot, excb[:, sl], tt, OP.add)
            nc.sync.dma_start(out=oexc_d[:, sl], in_=ot)

    nc.compile()
    return nc


_NC_CACHE = None


def _get_program():
    global _NC_CACHE
    if _NC_CACHE is None:
        _NC_CACHE = _build_program()
    return _NC_CACHE


def _build_in_maps(input_, inhibition, excitation,
                   aw_w, aw_b, au_w, au_b, iw_w, iw_b, iu_w, iu_b,
                   ew_w, ew_b, eu_w, eu_b,
                   ac_w, ac_b, ic_w, ic_b, ec_w, ec_b,
                   w_inh, w_exc, alpha, gamma, kappa, w_gain, mu,
                   bn0_w, bn0_b, bn1_w, bn1_b, step):
    f = lambda a: np.ascontiguousarray(np.asarray(a, dtype=np.float32))
    g = lambda a: np.ascontiguousarray(np.asarray(a, dtype=np.float16))
    stepf = float(np.asarray(step))

    x16 = g(input_).reshape(NCORES, H, SS)
    i16 = g(inhibition).reshape(NCORES, H, SS)
    e16 = g(excitation).reshape(NCORES, H, SS)

    # 1x1 weights, transposed to [I, O]; clock weights pre-scaled by step
    w1 = np.stack([
        f(aw_w).T, f(au_w).T, f(ac_w).T * stepf,
        f(iw_w).T, f(iu_w).T, f(ic_w).T * stepf,
        f(ew_w).T, f(eu_w).T, f(ec_w).T * stepf,
    ]).astype(np.float16)
    w5 = np.stack([
        f(w_inh).transpose(2, 3, 1, 0).reshape(25, H, H),
        f(w_exc).transpose(2, 3, 1, 0).reshape(25, H, H),
    ]).astype(np.float16)

    chan = lambda a: f(a).reshape(H)
    # clock biases (include +pi/2 for cos->sin shift), as K=1 matmul rows
    cb = np.stack([
        chan(ac_b) * stepf + np.pi / 2,
        chan(ic_b) * stepf + np.pi / 2,
        chan(ec_b) * stepf + np.pi / 2,
    ]).astype(np.float16)

    par = np.zeros((H, 16), dtype=np.float32)
    par[:, C_BATT] = chan(aw_b) + chan(au_b)
    par[:, C_BINH] = chan(iw_b) + chan(iu_b)
    par[:, C_BEXC] = chan(ew_b) + chan(eu_b)
    par[:, C_ALPHA] = chan(alpha)
    par[:, C_MU] = chan(mu)
    par[:, C_KAPPA] = chan(kappa)
    par[:, C_GAMMA] = chan(gamma)
    par[:, C_WGAIN] = chan(w_gain)
    par[:, C_BN0W] = chan(bn0_w)
    par[:, C_BN0B] = chan(bn0_b)
    par[:, C_BN1W] = chan(bn1_w)
    par[:, C_BN1B] = chan(bn1_b)

    in_maps = []
    for b in range(NCORES):
        in_maps.append({
            "x16": x16[b],
            "inh16": i16[b],
            "exc16": e16[b],
            "w1x1": w1,
            "w5": w5,
            "cbiasT": cb,
            "params": par,
        })
    return in_maps


def kernel(**inputs):
    in_maps = _build_in_maps(**inputs)
    nc = _get_program()
    res = run_bass_kernel_spmd(nc, in_maps, list(range(NCORES)))

    inh_new = np.stack([res.results[b]["out_inh"].reshape(H, S, S)
                        for b in range(NCORES)])
    exc_new = np.stack([res.results[b]["out_exc"].reshape(H, S, S)
                        for b in range(NCORES)])
    return inh_new.astype(np.float32), exc_new.astype(np.float32)
